# revision 40
# baseline (speedup 1.0000x reference)
"""Fused BasicTransformerBlock (self-attn + cross-attn + GEGLU FF) on 8
Trainium2 NeuronCores.

Sharding: data-parallel over batch (2) x 4-way token split within each batch
element; each core owns 512 tokens end-to-end. No collectives: every core
computes k/v for the full 2048-token sequence of its batch element from a
replicated fp8 copy of x (the host rotates the sequence per core so the own
block is always block 0 - attention is permutation-invariant over kv).

Precision plan (validated against the reference in numpy emulation,
rel-to-max ~1.0e-2 vs the 2e-2 gate):
- fp8(e4m3) DoubleRow matmuls (0.5 cyc/row, 2 contraction chunks per
  instruction) for q/k/v/q2 projections, attention scores, and attn@v.
  Those weights are pre-scaled by 64 on the host so ~N(0,0.02) entries stay
  in fp8's normal range; the 1/64 descale folds into the per-token rstd
  (sqrt computed with a 4096x scale) so no extra ops appear.
- Everything feeding the residual stream (out-projections, GEGLU, FF-out)
  and the small encoder k2/v2 GEMMs stay bf16: fp8 there pushes the error
  past the gate.
- Scores use a zero-padded DoubleRow: the moving q tile carries an all-zero
  second subtile, so the stationary k pair can be any adjacent finite bytes
  (the next kv-tile's k; a zeroed dummy chunk covers the last tile).
- Softmax runs without max-subtraction (scores are O(5) here); exp gets a
  constant -1 bias (softmax-invariant) so fp8 ex stays below e4m3 max. The
  denominator comes from a ones-column appended to v (head dim padded to 80
  so the fp8 DoubleRow pair stride stays 16B-aligned).

LayerNorm folds into the weights on the host (W' = g*W.T centered); only
per-token rstd survives on chip. LN1 stats come from the fp8 x copy via
DoubleRow ones-matmuls, LN2/LN3 from the bf16 residual.
"""

import numpy as np
import ml_dtypes

import concourse.bass as bass
import concourse.tile as tile
from concourse import bacc, mybir
from concourse.bass_utils import run_bass_kernel_spmd

BF16 = mybir.dt.bfloat16
F32 = mybir.dt.float32
FP8 = mybir.dt.float8e4
AF = mybir.ActivationFunctionType
ALU = mybir.AluOpType
DR = mybir.MatmulPerfMode.DoubleRow
NPBF16 = ml_dtypes.bfloat16
NPF8 = ml_dtypes.float8_e4m3

HID = 1280
KC = HID // 128            # 10 hid chunks
KP = KC // 2               # 5 hid chunk-pairs
T = 512                    # own tokens per core
S = 2048                   # full sequence (self-attn kv length)
NTB = S // T               # 4 token blocks of 512
ST = S // 128              # 16 kv tiles of 128
FC = 40                    # 5120/128
CROSS = 2048
CC = CROSS // 128          # 16
SE = 77                    # encoder sequence length
SEP = 80                   # padded
NH = 20
HD = 64
HDP = 80                   # head dim + ones col, padded to 16B multiple
EPS = 1e-5
WS = 64.0                  # fp8 weight pre-scale
SHIFT = 1.0                # exp bias shift (softmax-invariant)
N_CORES = 8

DBLOCKS = [(0, 8), (512, 8), (1024, 4)]
PHASE_MARKS = []


class _Pool:
    """Tile pool with manual open/close. Closes must be LIFO w.r.t. opens."""

    def __init__(self, tc, **kw):
        self._cm = tc.tile_pool(**kw)
        self.pool = self._cm.__enter__()
        self._n = 0

    def tile(self, *a, **kw):
        if "tag" not in kw:
            kw["tag"] = f"auto{self._n}"
            self._n += 1
        if "name" not in kw:
            kw["name"] = kw["tag"]
        return self.pool.tile(*a, **kw)

    def close(self):
        self._cm.__exit__(None, None, None)


def _emit(nc, tc, d, flags, pref):
    constp = _Pool(tc, name=f"{pref}const", bufs=1)
    dramp = _Pool(tc, name=f"{pref}dram", bufs=1, space="DRAM")

    ones8_t = constp.tile([128, 2, 16], FP8)    # DR-stats stationary
    nc.vector.memset(ones8_t, 1.0)              # pair stride 16B (ISA rule)
    ones8 = ones8_t[:, :, 0:1]
    ones_b = constp.tile([128, 1], BF16)
    nc.vector.memset(ones_b, 1.0)
    shift_t = constp.tile([128, 1], F32)        # exp bias
    nc.vector.memset(shift_t, -SHIFT)
    eps1_t = constp.tile([1, 1], F32)           # LN1/2: sqrt(4096*(var+eps))
    nc.vector.memset(eps1_t, 4096.0 * EPS)
    eps_t = constp.tile([1, 1], F32)            # LN3: sqrt(var + eps)
    nc.vector.memset(eps_t, EPS)

    def load_col(name):
        t_ = constp.tile(list(d[name].shape), F32, tag=name)
        nc.sync.dma_start(out=t_, in_=d[name][:, :])
        return t_

    bo1_sb = load_col("bo1c") if flags["bo1"] else None
    bo2_sb = load_col("bo2c") if flags["bo2"] else None
    bff_sb = load_col("bffc") if flags["bff"] else None
    cq1_sb = load_col("cq1c") if flags["cq1"] else None
    ck1_sb = load_col("ck1c") if flags["ck1"] else None
    cv1_sb = None
    if flags["cv1"]:
        cv1_sb = constp.tile([128, HID], F32, tag="cv1b")
        nc.sync.dma_start(out=cv1_sb, in_=d["cv1b"][:, :])
    cq2_sb = load_col("cq2c") if flags["cq2"] else None
    cg1_sb = load_col("cg1c") if flags["cg1"] else None
    cg2_sb = load_col("cg2c") if flags["cg2"] else None

    # ------------- long-lived pools (open order = reverse close order) -----
    p_long = _Pool(tc, name=f"{pref}long", bufs=1)
    resid = p_long.tile([128, KC, T], F32)
    resid_bf = p_long.tile([128, KC, T], BF16)
    resid8 = p_long.tile([128, KC, T], FP8)

    p_cross = _Pool(tc, name=f"{pref}cross", bufs=1)   # closes after phase F
    obf = p_cross.tile([128, KC, T], FP8)
    encb = p_cross.tile([128, CC, SEP], BF16)
    k2T8 = p_cross.tile([128, KC, 2, SEP], FP8)
    v2 = p_cross.tile([128, NH, HDP], FP8)             # partitions 0..79
    q2z = p_cross.tile([128, KC, 2, T], FP8)
    o2bf = p_cross.tile([128, KC, T], FP8)

    p_kv = _Pool(tc, name=f"{pref}kv", bufs=1)         # closes after phase C
    kT8 = p_kv.tile([128, KC + 1, S], FP8)             # +1 dummy chunk
    v8 = p_kv.tile([128, ST // 2, NH, 2, HDP], FP8)
    qz = p_kv.tile([128, KC, 2, T], FP8)

    p_x8 = _Pool(tc, name=f"{pref}x8", bufs=1)         # closes after phase B
    x8 = p_x8.tile([128, KC, S], FP8)

    kT8f = kT8.rearrange("p c n -> p (c n)")

    # ---------------- Phase A: loads, zero fills, LN1 stats ----------------
    nc.sync.dma_start(
        out=x8, in_=d["x8T"][:, :].rearrange("(c p) n -> p c n", p=128))

    nc.gpsimd.memset(kT8[:, KC, :], 0.0)
    nc.gpsimd.memset(qz[:, :, 1, :], 0.0)
    nc.gpsimd.memset(q2z[:, :, 1, :], 0.0)
    nc.vector.memset(v8[:, :, :, :, HD:HDP], 0.0)
    nc.vector.memset(v8[:, :, :, :, HD:HD + 1], 1.0)
    nc.vector.memset(k2T8[:, :, 1, :], 0.0)
    nc.vector.memset(v2.rearrange("p h e -> p (h e)"), 0.0)
    nc.vector.memset(v2[0:SE, :, HD:HD + 1], 1.0)

    p_rstd1 = _Pool(tc, name=f"{pref}rstd1", bufs=1)   # closes after phase B
    rstd1_bc = p_rstd1.tile([128, S], F32)
    rstd1_row = p_rstd1.tile([1, S], F32)
    rstd1_col = p_rstd1.tile([128, ST], F32)

    PHASE_MARKS.append(('A', nc.next_id()))
    pA = _Pool(tc, name=f"{pref}phA", bufs=2)
    psA = _Pool(tc, name=f"{pref}psA", bufs=2, space="PSUM")
    for tb in range(NTB):
        ts = slice(tb * T, (tb + 1) * T)
        psx = psA.tile([1, T], F32, tag="stx")
        psq = psA.tile([1, T], F32, tag="stq")
        for c in range(KP):
            sq8 = pA.tile([128, 2, T], FP8, tag="sq8", bufs=3)
            nc.scalar.square(out=sq8, in_=x8[:, 2 * c:2 * c + 2, ts])
            nc.tensor.matmul(psx[0:1, :], ones8, x8[:, 2 * c:2 * c + 2, ts],
                             start=(c == 0), stop=(c == KP - 1), perf_mode=DR)
            nc.tensor.matmul(psq[0:1, :], ones8, sq8,
                             start=(c == 0), stop=(c == KP - 1), perf_mode=DR)
        mu = pA.tile([1, T], F32, tag="mu", bufs=1)
        ex2 = pA.tile([1, T], F32, tag="ex2", bufs=1)
        nc.vector.tensor_scalar_mul(out=mu, in0=psx[0:1, :], scalar1=1.0 / HID)
        nc.vector.tensor_scalar_mul(out=ex2, in0=psq[0:1, :],
                                    scalar1=1.0 / HID)
        var = pA.tile([1, T], F32, tag="var", bufs=1)
        nc.vector.tensor_tensor(out=var, in0=mu, in1=mu, op=ALU.mult)
        nc.vector.tensor_sub(out=var, in0=ex2, in1=var)
        sd = pA.tile([1, T], F32, tag="sd", bufs=1)
        # sd = sqrt(4096*(var+eps)) = 64*sd_true -> rec = rstd/64 (descale)
        nc.scalar.activation(out=sd, in_=var, func=AF.Sqrt,
                             bias=eps1_t[0:1, 0:1], scale=4096.0)
        nc.vector.reciprocal(out=rstd1_row[0:1, ts], in_=sd)
    nc.gpsimd.partition_broadcast(rstd1_bc, rstd1_row[0:1, :])
    rdrt = dramp.tile([1, S], F32, tag="rbounce")
    nc.sync.dma_start(out=rdrt[:, :], in_=rstd1_row[0:1, :])
    nc.sync.dma_start(
        out=rstd1_col,
        in_=rdrt[0:1, :].rearrange("a (t p) -> (a p) t", p=128))
    psA.close()
    pA.close()

    # ---------------- Phase B: k (full seq), q (own), v (full seq) --------
    PHASE_MARKS.append(('B', nc.next_id()))
    pB = _Pool(tc, name=f"{pref}phB", bufs=3)
    psB = _Pool(tc, name=f"{pref}psB", bufs=4, space="PSUM")

    def w_dma8(pool, dram_t, ot, kch, tag, bufs=3):
        w_sb = pool.tile([128, kch, 128], FP8, tag=tag, bufs=bufs)
        nc.sync.dma_start(out=w_sb, in_=dram_t[ot, :, :, :])
        return w_sb

    def w_dmab(pool, dram_t, ot, kch, tag, bufs=3):
        w_sb = pool.tile([128, kch, 128], BF16, tag=tag, bufs=bufs)
        nc.sync.dma_start(out=w_sb, in_=dram_t[ot, :, :, :])
        return w_sb


    for ot in range(KC):
        wsb = w_dma8(pB, d["wk1t8"], ot, KC, "wk")
        for tb in range(NTB):
            ts = slice(tb * T, (tb + 1) * T)
            ps = psB.tile([128, T], F32, tag="psB")
            for c in range(KP):
                nc.tensor.matmul(ps[:, :], wsb[:, 2 * c:2 * c + 2, :],
                                 x8[:, 2 * c:2 * c + 2, ts],
                                 start=(c == 0), stop=(c == KP - 1),
                                 perf_mode=DR)
            if ck1_sb is None:
                nc.vector.tensor_tensor(out=kT8[:, ot, ts], in0=ps[:, :],
                                  in1=rstd1_bc[:, ts], op=ALU.mult)
            else:
                tmp = pB.tile([128, T], F32, tag="ktmp", bufs=2)
                nc.vector.tensor_tensor(out=tmp, in0=ps[:, :],
                                  in1=rstd1_bc[:, ts], op=ALU.mult)
                nc.vector.tensor_scalar_add(out=kT8[:, ot, ts], in0=tmp,
                                      scalar1=ck1_sb[:, ot:ot + 1])
    PHASE_MARKS.append(('B.q', nc.next_id()))
    # q for own tokens (block 0 after host rotation)
    for ot in range(KC):
        wsb = w_dma8(pB, d["wq1t8"], ot, KC, "wq")
        ps = psB.tile([128, T], F32, tag="psB")
        for c in range(KP):
            nc.tensor.matmul(ps[:, :], wsb[:, 2 * c:2 * c + 2, :],
                             x8[:, 2 * c:2 * c + 2, 0:T],
                             start=(c == 0), stop=(c == KP - 1), perf_mode=DR)
        if cq1_sb is None:
            nc.vector.tensor_tensor(out=qz[:, ot, 0, :], in0=ps[:, :],
                              in1=rstd1_bc[:, 0:T], op=ALU.mult)
        else:
            tmp = pB.tile([128, T], F32, tag="qtmp", bufs=2)
            nc.vector.tensor_tensor(out=tmp, in0=ps[:, :],
                              in1=rstd1_bc[:, 0:T], op=ALU.mult)
            nc.vector.tensor_scalar_add(out=qz[:, ot, 0, :], in0=tmp,
                                  scalar1=cq1_sb[:, ot:ot + 1])
    PHASE_MARKS.append(('B.v', nc.next_id()))
    # v for all 2048 tokens (token-major, padded kv-pair layout)
    wv_sb = pB.tile([128, KC, HID], FP8, tag="wv", bufs=1)
    nc.sync.dma_start(out=wv_sb, in_=d["wv1t8"][:, :, :])
    for tt in range(ST):
        j, par = divmod(tt, 2)
        for d0, nh in DBLOCKS:
            dn = nh * HD
            ps = psB.tile([128, T], F32, tag="psB")
            for c in range(KP):
                nc.tensor.matmul(
                    ps[:, 0:dn],
                    x8[:, 2 * c:2 * c + 2, tt * 128:(tt + 1) * 128],
                    wv_sb[:, 2 * c:2 * c + 2, d0:d0 + dn],
                    start=(c == 0), stop=(c == KP - 1), perf_mode=DR)
            dst = v8[:, j, d0 // HD:d0 // HD + nh, par, 0:HD]
            src = ps[:, 0:dn].rearrange("p (h e) -> p h e", e=HD)
            if cv1_sb is None:
                nc.scalar.activation(out=dst, in_=src, func=AF.Copy,
                                     scale=rstd1_col[:, tt:tt + 1])
            else:
                tmp = pB.tile([128, T], F32, tag="vtmp", bufs=2)
                nc.scalar.activation(
                    out=tmp[:, 0:dn].rearrange("p (h e) -> p h e", e=HD),
                    in_=src, func=AF.Copy, scale=rstd1_col[:, tt:tt + 1])
                nc.vector.tensor_add(
                    out=dst,
                    in0=tmp[:, 0:dn].rearrange("p (h e) -> p h e", e=HD),
                    in1=cv1_sb[:, d0:d0 + dn].rearrange("p (h e) -> p h e",
                                                        e=HD))
    psB.close()
    pB.close()
    p_rstd1.close()
    p_x8.close()

    # ---------------- Phase C: self-attention ----------------
    PHASE_MARKS.append(('C', nc.next_id()))
    pC = _Pool(tc, name=f"{pref}phC", bufs=4)
    psS = _Pool(tc, name=f"{pref}psS", bufs=2, space="PSUM")
    psO = _Pool(tc, name=f"{pref}psO", bufs=2, space="PSUM")

    for h in range(NH):
        hp, half = divmod(h, 2)
        base = HD * half
        po = psO.tile([128, T], F32, tag="psO")
        for j in range(ST // 2):
            ps = psS.tile([128, 2 * T], F32, tag="psS")
            for sub in range(2):
                tt = 2 * j + sub
                kap = kT8f[base:base + 64,
                           hp * S + tt * 128:hp * S + tt * 128 + 256]
                kap = kap.rearrange("p (a m) -> p a m", a=2)
                nc.tensor.matmul(ps[:, sub * T:(sub + 1) * T], kap,
                                 qz[base:base + 64, hp, :, :],
                                 start=True, stop=True, perf_mode=DR)
            ex8 = pC.tile([128, 2, T], FP8, tag="ex8", bufs=6)
            nc.scalar.activation(
                out=ex8.rearrange("p a n -> p (a n)"), in_=ps[:, :],
                func=AF.Exp, bias=shift_t[:, 0:1])
            nc.tensor.matmul(po[0:HDP, :], v8[:, j, h, :, :], ex8,
                             start=(j == 0), stop=(j == ST // 2 - 1),
                             perf_mode=DR)
        rec = pC.tile([1, T], BF16, tag="rec", bufs=2)
        with nc.allow_low_precision(reason="softmax denom bf16"):
            nc.vector.reciprocal(out=rec, in_=po[HD:HD + 1, :])
        rbc = pC.tile([HD, T], BF16, tag="rbc", bufs=2)
        nc.gpsimd.partition_broadcast(rbc, rec[0:1, :])
        with nc.allow_low_precision(reason="attention out fp8"):
            nc.vector.tensor_tensor(out=obf[base:base + HD, hp, :],
                                    in0=po[0:HD, :], in1=rbc, op=ALU.mult)
        if h == 3:
            cross_kv()
    psO.close()
    psS.close()
    pC.close()
    p_kv.close()

    # ---------------- Phase D: out-proj (bf16) + residual ----------------
    PHASE_MARKS.append(('D', nc.next_id()))
    pD = _Pool(tc, name=f"{pref}phD", bufs=3)
    psD = _Pool(tc, name=f"{pref}psD", bufs=4, space="PSUM")
    def cross_kv():
        # emitted inside the attention window: fills PE while ACT runs exp
        nc.sync.dma_start(
            out=encb,
            in_=d["encbT"][:, :].rearrange("(c p) n -> p c n", p=128))
        for ot in range(KC):
            wsb = w_dmab(pD, d["wk2t"], ot, CC, "wk2", bufs=2)
            ps = psD.tile([128, SEP], F32, tag="psk2", bufs=1)
            for c in range(CC):
                nc.tensor.matmul(ps[:, :], wsb[:, c, :], encb[:, c, :],
                                 start=(c == 0), stop=(c == CC - 1))
            with nc.allow_low_precision(reason="attention k2 fp8"):
                nc.vector.tensor_copy(out=k2T8[:, ot, 0, :], in_=ps[:, :])
        wv2_sb = pD.tile([128, CC, HID], BF16, tag="wv2", bufs=1)
        nc.sync.dma_start(out=wv2_sb, in_=d["wv2t"][:, :, :])
        for d0, nh in DBLOCKS:
            dn = nh * HD
            ps = psD.tile([128, T], F32, tag="psv2", bufs=1)
            for c in range(CC):
                nc.tensor.matmul(ps[0:SEP, 0:dn], encb[:, c, :],
                                 wv2_sb[:, c, d0:d0 + dn],
                                 start=(c == 0), stop=(c == CC - 1))
            with nc.allow_low_precision(reason="attention v2 fp8"):
                nc.vector.tensor_copy(
                    out=v2[0:SEP, d0 // HD:d0 // HD + nh, 0:HD],
                    in_=ps[0:SEP, 0:dn].rearrange("p (h e) -> p h e", e=HD))

    PHASE_MARKS.append(('C.heads', nc.next_id()))
    cross_kv()
    xown = pD.tile([128, KC, T], F32, tag="xown", bufs=1)
    nc.sync.dma_start(
        out=xown, in_=d["xT"][:, :].rearrange("(c p) n -> p c n", p=128))
    for ot in range(KC):
        wsb = w_dma8(pD, d["wo1t"], ot, KC, "wo")
        ps = psD.tile([128, T], F32, tag="psD")
        for c in range(KP):
            nc.tensor.matmul(ps[:, :], wsb[:, 2 * c:2 * c + 2, :],
                             obf[:, 2 * c:2 * c + 2, :],
                             start=(c == 0), stop=(c == KP - 1), perf_mode=DR)
        if bo1_sb is None:
            nc.vector.scalar_tensor_tensor(
                out=resid[:, ot, :], in0=ps[:, :], scalar=1.0 / WS,
                op0=ALU.mult, in1=xown[:, ot, :], op1=ALU.add)
        else:
            tmp = pD.tile([128, T], F32, tag="dtmp", bufs=2)
            nc.vector.tensor_scalar_mul(out=tmp, in0=ps[:, :],
                                        scalar1=1.0 / WS)
            nc.vector.scalar_tensor_tensor(
                out=resid[:, ot, :], in0=tmp,
                scalar=bo1_sb[:, ot:ot + 1], op0=ALU.add,
                in1=xown[:, ot, :], op1=ALU.add)
        nc.gpsimd.tensor_copy(out=resid_bf[:, ot, :], in_=resid[:, ot, :])
        with nc.allow_low_precision(reason="q2 gemm input fp8"):
            nc.vector.tensor_copy(out=resid8[:, ot, :], in_=resid[:, ot, :])
    psD.close()
    pD.close()

    # ---------------- LN stats helper (bf16, own tokens) -------------
    def ln_stats(statp, psp, sqtag, scaled):
        psx = psp.tile([1, T], F32, tag=f"{sqtag}px")
        psq = psp.tile([1, T], F32, tag=f"{sqtag}pq")
        for c in range(KC):
            sq = statp.tile([128, T], BF16, tag=f"{sqtag}sq", bufs=2)
            nc.scalar.square(out=sq, in_=resid_bf[:, c, :])
            nc.tensor.matmul(psx[0:1, :], ones_b[:, 0:1], resid_bf[:, c, :],
                             start=(c == 0), stop=(c == KC - 1))
            nc.tensor.matmul(psq[0:1, :], ones_b[:, 0:1], sq[:, :],
                             start=(c == 0), stop=(c == KC - 1))
        mu = statp.tile([1, T], F32, tag=f"{sqtag}mu")
        ex2 = statp.tile([1, T], F32, tag=f"{sqtag}ex2")
        nc.vector.tensor_scalar_mul(out=mu, in0=psx[0:1, :], scalar1=1.0 / HID)
        nc.vector.tensor_scalar_mul(out=ex2, in0=psq[0:1, :],
                                    scalar1=1.0 / HID)
        var = statp.tile([1, T], F32, tag=f"{sqtag}var")
        nc.vector.tensor_tensor(out=var, in0=mu, in1=mu, op=ALU.mult)
        nc.vector.tensor_sub(out=var, in0=ex2, in1=var)
        sd = statp.tile([1, T], F32, tag=f"{sqtag}sd")
        if scaled:
            nc.scalar.activation(out=sd, in_=var, func=AF.Sqrt,
                                 bias=eps1_t[0:1, 0:1], scale=4096.0)
        else:
            nc.scalar.activation(out=sd, in_=var, func=AF.Sqrt,
                                 bias=eps_t[0:1, 0:1])
        rstd = statp.tile([1, T], F32, tag=f"{sqtag}rstd")
        nc.vector.reciprocal(out=rstd, in_=sd)
        return rstd

    # ---------------- Phase E: LN2 stats ----------------
    p_rstd2 = _Pool(tc, name=f"{pref}rstd2", bufs=1)
    rstd2_bc = p_rstd2.tile([128, T], F32)
    PHASE_MARKS.append(('E', nc.next_id()))
    pE = _Pool(tc, name=f"{pref}phE", bufs=2)
    psE = _Pool(tc, name=f"{pref}psE", bufs=1, space="PSUM")
    rstd2_row = ln_stats(pE, psE, "e", scaled=True)
    nc.gpsimd.partition_broadcast(rstd2_bc, rstd2_row[0:1, :])
    psE.close()
    pE.close()

    # ---------------- Phase F: cross-attention ----------------
    PHASE_MARKS.append(('F', nc.next_id()))
    pF = _Pool(tc, name=f"{pref}phF", bufs=3)
    psF = _Pool(tc, name=f"{pref}psF", bufs=3, space="PSUM")

    for ot in range(KC):
        wsb = w_dma8(pF, d["wq2t8"], ot, KC, "wq2")
        ps = psF.tile([128, T], F32, tag="psF", bufs=2)
        for c in range(KP):
            nc.tensor.matmul(ps[:, :], wsb[:, 2 * c:2 * c + 2, :],
                             resid8[:, 2 * c:2 * c + 2, :],
                             start=(c == 0), stop=(c == KP - 1), perf_mode=DR)
        if cq2_sb is None:
            nc.vector.tensor_tensor(out=q2z[:, ot, 0, :], in0=ps[:, :],
                              in1=rstd2_bc, op=ALU.mult)
        else:
            tmp = pF.tile([128, T], F32, tag="q2tmp", bufs=2)
            nc.vector.tensor_tensor(out=tmp, in0=ps[:, :], in1=rstd2_bc,
                              op=ALU.mult)
            nc.vector.tensor_scalar_add(out=q2z[:, ot, 0, :], in0=tmp,
                                  scalar1=cq2_sb[:, ot:ot + 1])
    PHASE_MARKS.append(('F.heads', nc.next_id()))
    for h in range(NH):
        hp, half = divmod(h, 2)
        base = HD * half
        ps = psF.tile([128, T], F32, tag="ps2s", bufs=2)
        nc.tensor.matmul(ps[0:SEP, :], k2T8[base:base + 64, hp, :, :],
                         q2z[base:base + 64, hp, :, :],
                         start=True, stop=True, perf_mode=DR)
        ex2 = pF.tile([128, T], FP8, tag="ex2", bufs=3)
        nc.scalar.activation(out=ex2[0:SEP, :], in_=ps[0:SEP, :], func=AF.Exp,
                             bias=shift_t[0:SEP, 0:1])
        po = psF.tile([128, T], F32, tag="ps2o", bufs=2)
        nc.tensor.matmul(po[0:HDP, :], v2[0:SE, h, :], ex2[0:SE, :],
                         start=True, stop=True)
        rec = pF.tile([1, T], BF16, tag="rec2", bufs=2)
        with nc.allow_low_precision(reason="softmax denom bf16"):
            nc.vector.reciprocal(out=rec, in_=po[HD:HD + 1, :])
        rbc = pF.tile([HD, T], BF16, tag="rbc2", bufs=2)
        nc.gpsimd.partition_broadcast(rbc, rec[0:1, :])
        with nc.allow_low_precision(reason="attention out fp8"):
            nc.vector.tensor_tensor(out=o2bf[base:base + HD, hp, :],
                                    in0=po[0:HD, :], in1=rbc, op=ALU.mult)
    PHASE_MARKS.append(('F.oproj', nc.next_id()))
    for ot in range(KC):
        wsb = w_dma8(pF, d["wo2t"], ot, KC, "wo2")
        ps = psF.tile([128, T], F32, tag="psF", bufs=2)
        for c in range(KP):
            nc.tensor.matmul(ps[:, :], wsb[:, 2 * c:2 * c + 2, :],
                             o2bf[:, 2 * c:2 * c + 2, :],
                             start=(c == 0), stop=(c == KP - 1), perf_mode=DR)
        if bo2_sb is None:
            nc.vector.scalar_tensor_tensor(
                out=resid[:, ot, :], in0=ps[:, :], scalar=1.0 / WS,
                op0=ALU.mult, in1=resid[:, ot, :], op1=ALU.add)
        else:
            tmp = pF.tile([128, T], F32, tag="ftmp", bufs=2)
            nc.vector.tensor_scalar_mul(out=tmp, in0=ps[:, :],
                                        scalar1=1.0 / WS)
            nc.vector.scalar_tensor_tensor(
                out=resid[:, ot, :], in0=tmp,
                scalar=bo2_sb[:, ot:ot + 1], op0=ALU.add,
                in1=resid[:, ot, :], op1=ALU.add)
        nc.gpsimd.tensor_copy(out=resid_bf[:, ot, :], in_=resid[:, ot, :])
    psF.close()
    pF.close()
    p_rstd2.close()
    p_cross.close()

    # ---------------- Phase G: LN3 stats ----------------
    p_gT = _Pool(tc, name=f"{pref}gT", bufs=1)
    gT_bf = p_gT.tile([128, FC, T], BF16)
    rstd3_bc = p_gT.tile([128, T], F32)
    PHASE_MARKS.append(('G', nc.next_id()))
    pG = _Pool(tc, name=f"{pref}phG", bufs=2)
    psG = _Pool(tc, name=f"{pref}psG", bufs=1, space="PSUM")
    rstd3_row = ln_stats(pG, psG, "g", scaled=False)
    nc.gpsimd.partition_broadcast(rstd3_bc, rstd3_row[0:1, :])
    psG.close()
    pG.close()

    # ---------------- Phase H: GEGLU (bf16) ----------------
    PHASE_MARKS.append(('H', nc.next_id()))
    pH = _Pool(tc, name=f"{pref}phH", bufs=3)
    psH = _Pool(tc, name=f"{pref}psH", bufs=4, space="PSUM")
    for j in range(FC):
        w1 = w_dmab(pH, d["wgt"], j, KC, "wg1")
        w2 = w_dmab(pH, d["wgt"], FC + j, KC, "wg2")
        ps1 = psH.tile([128, T], F32, tag="psH")
        ps2 = psH.tile([128, T], F32, tag="psH")
        for c in range(KC):
            nc.tensor.matmul(ps1[:, :], w1[:, c, :], resid_bf[:, c, :],
                             start=(c == 0), stop=(c == KC - 1))
            nc.tensor.matmul(ps2[:, :], w2[:, c, :], resid_bf[:, c, :],
                             start=(c == 0), stop=(c == KC - 1))
        u2 = pH.tile([128, T], F32, tag="u2", bufs=3)
        nc.vector.tensor_tensor(out=u2, in0=ps2[:, :], in1=rstd3_bc,
                                op=ALU.mult)
        if cg2_sb is not None:
            nc.gpsimd.tensor_scalar_add(out=u2, in0=u2,
                                        scalar1=cg2_sb[:, j:j + 1])
        g2 = pH.tile([128, T], BF16, tag="g2", bufs=3)
        nc.scalar.activation(out=g2, in_=u2, func=AF.Gelu)
        u1 = pH.tile([128, T], BF16, tag="u1", bufs=3)
        nc.vector.tensor_tensor(out=u1, in0=ps1[:, :], in1=rstd3_bc,
                                op=ALU.mult)
        if cg1_sb is not None:
            nc.vector.tensor_scalar_add(out=u1, in0=u1,
                                        scalar1=cg1_sb[:, j:j + 1])
        nc.vector.tensor_tensor(out=gT_bf[:, j, :], in0=u1, in1=g2,
                                op=ALU.mult)
    psH.close()
    pH.close()

    # ---------------- Phase I: FF out (bf16) + residual ----------------
    PHASE_MARKS.append(('I', nc.next_id()))
    pI = _Pool(tc, name=f"{pref}phI", bufs=2)
    psI = _Pool(tc, name=f"{pref}psI", bufs=3, space="PSUM")
    for ot in range(KC):
        wsb = pI.tile([128, FC, 128], BF16, tag="wf", bufs=3)
        nc.sync.dma_start(out=wsb, in_=d["wft"][ot, :, :, :])
        ps = psI.tile([128, T], F32, tag="psI")
        for c in range(FC):
            nc.tensor.matmul(ps[:, :], wsb[:, c, :], gT_bf[:, c, :],
                             start=(c == 0), stop=(c == FC - 1))
        of = pI.tile([128, T], F32, tag="of", bufs=2)
        if bff_sb is None:
            nc.vector.tensor_add(out=of, in0=ps[:, :], in1=resid[:, ot, :])
        else:
            nc.vector.scalar_tensor_tensor(
                out=of, in0=ps[:, :], scalar=bff_sb[:, ot:ot + 1],
                op0=ALU.add, in1=resid[:, ot, :], op1=ALU.add)
        nc.sync.dma_start(out=d["outT"][ot * 128:(ot + 1) * 128, :], in_=of)
    psI.close()
    pI.close()
    p_gT.close()
    p_long.close()

    dramp.close()
    constp.close()


def _build(flags):
    nc = bacc.Bacc("TRN2", target_bir_lowering=False, num_devices=N_CORES)
    d = {}
    d["x8T"] = nc.dram_tensor("x8T", [HID, S], FP8, kind="ExternalInput")
    d["xT"] = nc.dram_tensor("xT", [HID, T], F32, kind="ExternalInput")
    d["encbT"] = nc.dram_tensor("encbT", [CROSS, SEP], BF16,
                                kind="ExternalInput")
    for n in ["wq1t8", "wk1t8", "wq2t8"]:
        d[n] = nc.dram_tensor(n, [KC, 128, KC, 128], FP8, kind="ExternalInput")
    d["wv1t8"] = nc.dram_tensor("wv1t8", [128, KC, HID], FP8,
                                kind="ExternalInput")
    d["wk2t"] = nc.dram_tensor("wk2t", [KC, 128, CC, 128], BF16,
                               kind="ExternalInput")
    d["wv2t"] = nc.dram_tensor("wv2t", [128, CC, HID], BF16,
                               kind="ExternalInput")
    for n in ["wo1t", "wo2t"]:
        d[n] = nc.dram_tensor(n, [KC, 128, KC, 128], FP8,
                              kind="ExternalInput")
    d["wgt"] = nc.dram_tensor("wgt", [2 * FC, 128, KC, 128], BF16,
                              kind="ExternalInput")
    d["wft"] = nc.dram_tensor("wft", [KC, 128, FC, 128], BF16,
                              kind="ExternalInput")
    for n, fl in [("bo1c", "bo1"), ("bo2c", "bo2"), ("bffc", "bff"),
                  ("cq1c", "cq1"), ("ck1c", "ck1"), ("cq2c", "cq2")]:
        if flags[fl]:
            d[n] = nc.dram_tensor(n, [128, KC], F32, kind="ExternalInput")
    if flags["cv1"]:
        d["cv1b"] = nc.dram_tensor("cv1b", [128, HID], F32,
                                   kind="ExternalInput")
    for n, fl in [("cg1c", "cg1"), ("cg2c", "cg2")]:
        if flags[fl]:
            d[n] = nc.dram_tensor(n, [128, FC], F32, kind="ExternalInput")
    d["outT"] = nc.dram_tensor("outT", [HID, T], F32, kind="ExternalOutput")

    with tile.TileContext(nc) as tc:
        _emit(nc, tc, d, flags, pref="r0_")
    nc.compile()
    return nc


def _colify(v, nch):
    return np.ascontiguousarray(np.asarray(v, np.float32).reshape(nch, 128).T)


def _prep(inputs):
    f32 = np.float32
    x = np.asarray(inputs["x"], f32)
    enc = np.asarray(inputs["encoder_hidden_states"], f32)
    g1, b1 = np.asarray(inputs["ln1_g"], f32), np.asarray(inputs["ln1_b"], f32)
    g2, b2 = np.asarray(inputs["ln2_g"], f32), np.asarray(inputs["ln2_b"], f32)
    g3, b3 = np.asarray(inputs["ln3_g"], f32), np.asarray(inputs["ln3_b"], f32)

    def foldT(w, g, scale=1.0):
        return (np.asarray(w, f32) * g[None, :]).T * scale   # [K, O]

    def center(wp):
        return wp - wp.mean(0, keepdims=True)

    def tobf(a):
        return np.ascontiguousarray(a.astype(NPBF16))

    def tof8(a):
        return np.ascontiguousarray(a.astype(NPF8))

    def stat4(wp, kch, nots, q):
        # [K, O] -> [n_ot, 128, kch, 128]: per-otile weight DMA is contiguous
        return q(wp.reshape(kch, 128, nots, 128).transpose(2, 1, 0, 3))

    def mov3(wp, kch, q):
        # [K, O] -> [128, kch, O] for moving-operand weight loads
        return q(wp.reshape(kch, 128, -1).transpose(1, 0, 2))

    scale = HD ** -0.5
    common = {}
    wq1p = foldT(inputs["wq1"], g1, scale)
    wk1p = foldT(inputs["wk1"], g1)
    wv1p = foldT(inputs["wv1"], g1)
    common["wq1t8"] = stat4(center(wq1p) * WS, KC, KC, tof8)
    common["wk1t8"] = stat4(center(wk1p) * WS, KC, KC, tof8)
    common["wv1t8"] = mov3(center(wv1p) * WS, KC, tof8)
    common["wo1t"] = stat4(np.asarray(inputs["wo1"], f32).T * WS, KC, KC,
                           tof8)
    wq2p = foldT(inputs["wq2"], g2, scale)
    common["wq2t8"] = stat4(center(wq2p) * WS, KC, KC, tof8)
    common["wk2t"] = stat4(np.asarray(inputs["wk2"], f32).T, CC, KC, tobf)
    common["wv2t"] = mov3(np.asarray(inputs["wv2"], f32).T, CC, tobf)
    common["wo2t"] = stat4(np.asarray(inputs["wo2"], f32).T * WS, KC, KC,
                           tof8)
    wgp = foldT(inputs["w_geglu"], g3)
    common["wgt"] = stat4(center(wgp), KC, 2 * FC, tobf)
    common["wft"] = stat4(np.asarray(inputs["w_ffout"], f32).T, FC, KC, tobf)

    encbT = np.zeros((2, CROSS, SEP), NPBF16)
    for b in range(2):
        encbT[b, :, :SE] = enc[b].T.astype(NPBF16)

    cq1 = b1 @ wq1p
    ck1 = b1 @ wk1p
    cv1 = b1 @ wv1p
    cq2 = b2 @ wq2p
    cg = b3 @ wgp + np.asarray(inputs["b_geglu"], f32)
    flags = {
        "cq1": not np.allclose(cq1, 0.0),
        "ck1": not np.allclose(ck1, 0.0),
        "cv1": not np.allclose(cv1, 0.0),
        "cq2": not np.allclose(cq2, 0.0),
        "cg1": not np.allclose(cg[:5120], 0.0),
        "cg2": not np.allclose(cg[5120:], 0.0),
        "bo1": not np.allclose(inputs["bo1"], 0.0),
        "bo2": not np.allclose(inputs["bo2"], 0.0),
        "bff": not np.allclose(inputs["b_ffout"], 0.0),
    }
    if flags["bo1"]:
        common["bo1c"] = _colify(inputs["bo1"], KC)
    if flags["bo2"]:
        common["bo2c"] = _colify(inputs["bo2"], KC)
    if flags["bff"]:
        common["bffc"] = _colify(inputs["b_ffout"], KC)
    if flags["cq1"]:
        common["cq1c"] = _colify(cq1, KC)
    if flags["ck1"]:
        common["ck1c"] = _colify(ck1, KC)
    if flags["cv1"]:
        common["cv1b"] = np.ascontiguousarray(
            np.broadcast_to(cv1[None, :], (128, HID)))
    if flags["cq2"]:
        common["cq2c"] = _colify(cq2, KC)
    if flags["cg1"]:
        common["cg1c"] = _colify(cg[:5120], FC)
    if flags["cg2"]:
        common["cg2c"] = _colify(cg[5120:], FC)

    per_core = []
    for core in range(N_CORES):
        b, r = divmod(core, 4)
        xT = np.ascontiguousarray(x[b, r * T:(r + 1) * T, :].T)
        # rotate the sequence so this core's own block is block 0: attention
        # sums over all kv tokens, so kv order is irrelevant
        x8T = np.ascontiguousarray(
            np.roll(x[b].T, -r * T, axis=1).astype(NPF8))
        per_core.append({"xT": xT, "x8T": x8T, "encbT": encbT[b]})
    return common, per_core, flags


_CACHE = {}


def _get_nc(flags):
    key = tuple(sorted(flags.items()))
    if key not in _CACHE:
        _CACHE[key] = _build(flags)
    return _CACHE[key]


def kernel(**inputs):
    common, per_core, flags = _prep(inputs)
    nc = _get_nc(flags)
    in_maps = [{**common, **pc} for pc in per_core]
    res = run_bass_kernel_spmd(nc, in_maps, core_ids=list(range(N_CORES)))
    out = np.empty((2, S, HID), np.float32)
    for core in range(N_CORES):
        b, r = divmod(core, 4)
        out[b, r * T:(r + 1) * T, :] = res.results[core]["outT"].T
    return out
